# revision 5
# baseline (speedup 1.0000x reference)
"""DecorrelatedBatchNorm on 8 trn2 NeuronCores.

Strategy (data-parallel over batch, per sharding hint):
  - shard x (64,56,56,256) -> rows of (200704, 256), 25088 rows/core
  - launch 1: per-core Gram matrix G_i = x_i^T x_i (256x256) on PE,
    float32r matmuls accumulating into PSUM; host sums the 8 partials.
  - host: mean, covariance, Cholesky, W = L^-1 in float64 (256x256 is
    trivial on CPU), folded with gamma/beta into  out = x @ A + b'
    where A = (diag(gamma) W)^T, b' = beta - gamma*(W m).
  - launch 2: out = x + (x @ E + b')  with E = A - I.  The identity split
    keeps the dominant term exact fp32 (E is small since cov ~ I), so the
    bf16 correction matmul contributes only ~1e-4 error.  The bias rides
    the correction PSUM via a constant outer-product matmul
    (ones/128 @ bias_bcast), leaving a single DVE add per subtile.
"""

import numpy as np
import ml_dtypes

import concourse.bass as bass
import concourse.tile as tile
from concourse import bacc, mybir
from concourse.bass_utils import run_bass_kernel_spmd
from concourse.masks import make_identity

B, W, H, C = 64, 56, 56, 256
N = B * W * H            # 200704 rows
NCORES = 8
NL = N // NCORES         # 25088 rows per core
F32 = mybir.dt.float32
F32R = mybir.dt.float32r
BF16 = mybir.dt.bfloat16
EPS = 0.001

SUBS_FULL = 16                     # 256-col subtiles per full chunk
CHUNKS = [SUBS_FULL] * 12 + [4]    # 12*2048 + 512 = 25088 rows

# test.py reads these for HW timing; harmless at grading time.
LAST_RESULTS = []


def _chunk_ap(t, row0, nsub):
    """Rows [row0, row0+128*nsub) of a (rows, C) DRAM tensor as a
    (128, nsub*C) access pattern; partition p holds rows row0+p*nsub..+nsub-1,
    so subtile s = [:, s*C:(s+1)*C] is a (128 rows, C ch) tile."""
    return t[row0:row0 + 128 * nsub, :].rearrange("(p b) c -> p (b c)", p=128)


def build_pass1():
    nc = bacc.Bacc(trn_type="TRN2", target_bir_lowering=False)
    x = nc.dram_tensor("x", [NL, C], F32, kind="ExternalInput").ap()
    g = nc.dram_tensor("g", [C, C], F32, kind="ExternalOutput").ap()
    nsubs_total = sum(CHUNKS)
    with tile.TileContext(nc) as tc:
        with (
            tc.tile_pool(name="xin", bufs=4) as xin,
            tc.tile_pool(name="acc", bufs=1, space="PSUM") as accp,
            tc.tile_pool(name="gout", bufs=1) as gout,
        ):
            g1 = accp.tile([128, C], F32)   # G rows 0..127
            g2 = accp.tile([128, C], F32)   # G rows 128..255
            si = 0
            row0 = 0
            for nsub in CHUNKS:
                xt = xin.tile([128, SUBS_FULL * C], F32R, tag="xt")
                nc.sync.dma_start(
                    out=xt[:, : nsub * C],
                    in_=_chunk_ap(x, row0, nsub).bitcast(F32R),
                )
                for s in range(nsub):
                    sub = xt[:, s * C:(s + 1) * C]
                    first = si == 0
                    last = si == nsubs_total - 1
                    nc.tensor.matmul(g1, sub[:, 0:128], sub, start=first, stop=last)
                    nc.tensor.matmul(g2, sub[:, 128:256], sub, start=first, stop=last)
                    si += 1
                row0 += 128 * nsub
            gs = gout.tile([128, 2 * C], F32)
            nc.scalar.copy(out=gs[:, 0:C], in_=g1)
            nc.scalar.copy(out=gs[:, C:2 * C], in_=g2)
            nc.sync.dma_start(
                out=g.rearrange("(a p) c -> p a c", p=128),
                in_=gs.rearrange("p (a c) -> p a c", a=2),
            )
    nc.finalize()
    return nc


def build_pass2():
    nc = bacc.Bacc(trn_type="TRN2", target_bir_lowering=False)
    x = nc.dram_tensor("x", [NL, C], F32, kind="ExternalInput").ap()
    e = nc.dram_tensor("e", [C, C], BF16, kind="ExternalInput").ap()
    bvec = nc.dram_tensor("bvec", [1, C], BF16, kind="ExternalInput").ap()
    y = nc.dram_tensor("y", [NL, C], F32, kind="ExternalOutput").ap()
    with tile.TileContext(nc) as tc:
        with (
            tc.tile_pool(name="xin", bufs=3) as xin,
            tc.tile_pool(name="xbf", bufs=2) as xbfp,
            tc.tile_pool(name="yout", bufs=3) as yout,
            tc.tile_pool(name="xtsb", bufs=3) as xtsb,
            tc.tile_pool(name="single", bufs=1) as single,
            tc.tile_pool(name="pst", bufs=3, space="PSUM") as pst,
            tc.tile_pool(name="psd", bufs=3, space="PSUM") as psd,
        ):
            ident = single.tile([128, 128], BF16)
            make_identity(nc, ident)
            # ones/128 stationary for the bias outer-product: summing K=128
            # copies of bias/128 reproduces bias exactly.
            ones_sc = single.tile([128, 128], BF16)
            nc.vector.memset(ones_sc, 1.0 / 128.0)
            # E as two K-blocks: e_sb[:, 0:C] = rows 0..127, [:, C:2C] = rows 128..255
            e_sb = single.tile([128, 2 * C], BF16)
            nc.sync.dma_start(out=e_sb.rearrange("p (a c) -> p a c", a=2),
                              in_=e.rearrange("(a p) c -> p a c", p=128))
            bias_sb = single.tile([128, C], BF16)
            bias_bc = bass.AP(
                tensor=bvec.tensor, offset=bvec.offset,
                ap=[[0, 128], [1, C]],
            )
            nc.gpsimd.dma_start(out=bias_sb, in_=bias_bc)

            row0 = 0
            for nsub in CHUNKS:
                xt = xin.tile([128, SUBS_FULL * C], F32, tag="xt")
                nc.sync.dma_start(out=xt[:, : nsub * C], in_=_chunk_ap(x, row0, nsub))
                xb = xbfp.tile([128, SUBS_FULL * C], BF16, tag="xb")
                nc.vector.tensor_copy(out=xb[:, : nsub * C], in_=xt[:, : nsub * C])
                yt = yout.tile([128, SUBS_FULL * C], F32, tag="yt")
                for s in range(nsub):
                    sub = xt[:, s * C:(s + 1) * C]
                    bsub = xb[:, s * C:(s + 1) * C]
                    # x^T (bf16) for this subtile: partitions=channels, free=rows
                    pxT = pst.tile([128, C], BF16, tag="pxT")
                    nc.tensor.transpose(pxT[:, 0:128], bsub[:, 0:128], ident)
                    nc.tensor.transpose(pxT[:, 128:256], bsub[:, 128:256], ident)
                    xT = xtsb.tile([128, C], BF16, tag="xT")
                    nc.scalar.copy(out=xT, in_=pxT)
                    # pd = bias + x @ E  (bias first via const outer product)
                    pd = psd.tile([128, C], F32, tag="pd")
                    nc.tensor.matmul(pd, ones_sc, bias_sb, start=True, stop=False)
                    nc.tensor.matmul(
                        pd, xT[:, 0:128], e_sb[:, 0:C], start=False, stop=False,
                    )
                    nc.tensor.matmul(
                        pd, xT[:, 128:256], e_sb[:, C:2 * C], start=False, stop=True,
                    )
                    ysub = yt[:, s * C:(s + 1) * C]
                    nc.vector.tensor_add(out=ysub, in0=sub, in1=pd)
                # stores ride the ACT HWDGE ring so they don't FIFO behind loads
                nc.scalar.dma_start(out=_chunk_ap(y, row0, nsub), in_=yt[:, : nsub * C])
                row0 += 128 * nsub
    nc.finalize()
    return nc


_PROGRAMS = {}


def _get_programs():
    if "p1" not in _PROGRAMS:
        _PROGRAMS["p1"] = build_pass1()
        _PROGRAMS["p2"] = build_pass2()
    return _PROGRAMS["p1"], _PROGRAMS["p2"]


def _tri_inv_lower(L):
    try:
        from scipy.linalg import solve_triangular
        return solve_triangular(L, np.eye(C, dtype=L.dtype), lower=True)
    except ImportError:
        return np.linalg.solve(L, np.eye(C, dtype=L.dtype))


def kernel(x, gamma, beta):
    LAST_RESULTS.clear()
    x = np.ascontiguousarray(x, dtype=np.float32)
    gamma = np.asarray(gamma, dtype=np.float64).reshape(C)
    beta = np.asarray(beta, dtype=np.float64).reshape(C)
    xf = x.reshape(N, C)
    nc1, nc2 = _get_programs()
    core_ids = list(range(NCORES))

    in_maps1 = [{"x": xf[i * NL:(i + 1) * NL]} for i in range(NCORES)]
    r1 = run_bass_kernel_spmd(nc1, in_maps1, core_ids=core_ids)
    LAST_RESULTS.append(("gram", r1))

    G = np.zeros((C, C), np.float64)
    for r in r1.results:
        G += r["g"].astype(np.float64)
    m = xf.sum(axis=0, dtype=np.float64) / N
    cov = (G - N * np.outer(m, m)) / (N - 1.0)
    ff = (1.0 - EPS) * cov + EPS * np.eye(C)
    L = np.linalg.cholesky(ff)
    Winv = _tri_inv_lower(L)                     # W = L^-1 (lower)
    A = Winv.T * gamma[None, :]                  # A[i,j] = gamma_j * W[j,i]
    E = np.ascontiguousarray((A - np.eye(C)).astype(ml_dtypes.bfloat16))
    bvec = np.ascontiguousarray(
        (beta - gamma * (Winv @ m)).astype(ml_dtypes.bfloat16).reshape(1, C)
    )

    in_maps2 = [
        {"x": xf[i * NL:(i + 1) * NL], "e": E, "bvec": bvec}
        for i in range(NCORES)
    ]
    r2 = run_bass_kernel_spmd(nc2, in_maps2, core_ids=core_ids)
    LAST_RESULTS.append(("whiten", r2))

    out = np.empty((N, C), np.float32)
    for i, r in enumerate(r2.results):
        out[i * NL:(i + 1) * NL] = r["y"]
    return out.reshape(B, W, H, C)


# revision 7
# speedup vs baseline: 1.0983x; 1.0983x over previous
"""DecorrelatedBatchNorm on 8 trn2 NeuronCores.

Strategy (data-parallel over batch, per sharding hint):
  - shard x (64,56,56,256) -> rows of (200704, 256), 25088 rows/core
  - launch 1: per-core Gram matrix G_i = x_i^T x_i (256x256) on PE,
    float32r matmuls accumulating into PSUM; host sums the 8 partials.
  - host: mean, covariance, Cholesky, W = L^-1 in float64 (256x256 is
    trivial on CPU), folded with gamma/beta into  out = x @ A + b'
    where A = (diag(gamma) W)^T, b' = beta - gamma*(W m).
  - launch 2: out = x + (x @ E + b')  with E = A - I.  The identity split
    keeps the dominant term exact fp32 (E is small since cov ~ I), so the
    bf16 correction matmul contributes only ~1e-4 error.  The bias rides
    the correction PSUM via a constant outer-product matmul
    (ones/128 @ bias_bcast), leaving a single DVE add per subtile.
"""

import numpy as np
import ml_dtypes

import concourse.bass as bass
import concourse.tile as tile
from concourse import bacc, mybir
from concourse.bass_utils import run_bass_kernel_spmd
from concourse.masks import make_identity

B, W, H, C = 64, 56, 56, 256
N = B * W * H            # 200704 rows
NCORES = 8
NL = N // NCORES         # 25088 rows per core
F32 = mybir.dt.float32
F32R = mybir.dt.float32r
BF16 = mybir.dt.bfloat16
EPS = 0.001

SUBS_FULL = 8                      # 256-col subtiles per full chunk
CHUNKS = [SUBS_FULL] * 24 + [4]    # 24*1024 + 512 = 25088 rows
GROUP = 4                          # subtiles per ACT/DVE/bias group

# test.py reads these for HW timing; harmless at grading time.
LAST_RESULTS = []


def _chunk_ap(t, row0, nsub):
    """Rows [row0, row0+128*nsub) of a (rows, C) DRAM tensor as a
    (128, nsub*C) access pattern; partition p holds rows row0+p*nsub..+nsub-1,
    so subtile s = [:, s*C:(s+1)*C] is a (128 rows, C ch) tile."""
    return t[row0:row0 + 128 * nsub, :].rearrange("(p b) c -> p (b c)", p=128)


def build_pass1():
    nc = bacc.Bacc(trn_type="TRN2", target_bir_lowering=False)
    x = nc.dram_tensor("x", [NL, C], F32, kind="ExternalInput").ap()
    g = nc.dram_tensor("g", [C, C], F32, kind="ExternalOutput").ap()
    nsubs_total = sum(CHUNKS)
    with tile.TileContext(nc) as tc:
        with (
            tc.tile_pool(name="xin", bufs=5) as xin,
            tc.tile_pool(name="acc", bufs=1, space="PSUM") as accp,
            tc.tile_pool(name="gout", bufs=1) as gout,
        ):
            g1 = accp.tile([128, C], F32)   # G rows 0..127
            g2 = accp.tile([128, C], F32)   # G rows 128..255
            si = 0
            row0 = 0
            for nsub in CHUNKS:
                xt = xin.tile([128, SUBS_FULL * C], F32R, tag="xt")
                nc.sync.dma_start(
                    out=xt[:, : nsub * C],
                    in_=_chunk_ap(x, row0, nsub).bitcast(F32R),
                )
                for s in range(nsub):
                    sub = xt[:, s * C:(s + 1) * C]
                    first = si == 0
                    last = si == nsubs_total - 1
                    nc.tensor.matmul(g1, sub[:, 0:128], sub, start=first, stop=last)
                    nc.tensor.matmul(g2, sub[:, 128:256], sub, start=first, stop=last)
                    si += 1
                row0 += 128 * nsub
            gs = gout.tile([128, 2 * C], F32)
            nc.scalar.copy(out=gs[:, 0:C], in_=g1)
            nc.scalar.copy(out=gs[:, C:2 * C], in_=g2)
            nc.sync.dma_start(
                out=g.rearrange("(a p) c -> p a c", p=128),
                in_=gs.rearrange("p (a c) -> p a c", a=2),
            )
    nc.finalize()
    return nc


def build_pass2():
    nc = bacc.Bacc(trn_type="TRN2", target_bir_lowering=False)
    x = nc.dram_tensor("x", [NL, C], F32, kind="ExternalInput").ap()
    e = nc.dram_tensor("e", [C, C], BF16, kind="ExternalInput").ap()
    bvec = nc.dram_tensor("bvec", [1, C], BF16, kind="ExternalInput").ap()
    y = nc.dram_tensor("y", [NL, C], F32, kind="ExternalOutput").ap()
    with tile.TileContext(nc) as tc:
        with (
            tc.tile_pool(name="xin", bufs=3) as xin,
            tc.tile_pool(name="xbf", bufs=2) as xbfp,
            tc.tile_pool(name="yout", bufs=3) as yout,
            tc.tile_pool(name="xtsb", bufs=3) as xtsb,
            tc.tile_pool(name="single", bufs=1) as single,
            tc.tile_pool(name="pst", bufs=2, space="PSUM") as pst,
            tc.tile_pool(name="psd", bufs=2, space="PSUM") as psd,
        ):
            ident = single.tile([128, 128], BF16)
            make_identity(nc, ident)
            # ones/128 stationary for the bias outer-product: summing K=128
            # copies of bias/128 reproduces bias exactly.
            ones_sc = single.tile([128, 128], BF16)
            nc.vector.memset(ones_sc, 1.0 / 128.0)
            # E as two K-blocks: e_sb[:, 0:C] = rows 0..127, [:, C:2C] = rows 128..255
            e_sb = single.tile([128, 2 * C], BF16)
            nc.sync.dma_start(out=e_sb.rearrange("p (a c) -> p a c", a=2),
                              in_=e.rearrange("(a p) c -> p a c", p=128))
            # bias replicated GROUP times so one outer-product MM covers a group
            bias_sb = single.tile([128, GROUP * C], BF16)
            bias_bc = bass.AP(
                tensor=bvec.tensor, offset=bvec.offset,
                ap=[[0, 128], [0, GROUP], [1, C]],
            )
            nc.gpsimd.dma_start(
                out=bias_sb.rearrange("p (g c) -> p g c", g=GROUP), in_=bias_bc)

            row0 = 0
            for nsub in CHUNKS:
                xt = xin.tile([128, SUBS_FULL * C], F32, tag="xt")
                nc.sync.dma_start(out=xt[:, : nsub * C], in_=_chunk_ap(x, row0, nsub))
                xb = xbfp.tile([128, SUBS_FULL * C], BF16, tag="xb")
                nc.vector.tensor_copy(out=xb[:, : nsub * C], in_=xt[:, : nsub * C])
                yt = yout.tile([128, SUBS_FULL * C], F32, tag="yt")
                for g0 in range(0, nsub, GROUP):
                    gc = GROUP * C
                    # x^T (bf16) for 4 subtiles into one psum bank
                    pxT = pst.tile([128, gc], BF16, tag="pxT")
                    for s in range(GROUP):
                        bsub = xb[:, (g0 + s) * C:(g0 + s + 1) * C]
                        nc.tensor.transpose(
                            pxT[:, s * C:s * C + 128], bsub[:, 0:128], ident)
                        nc.tensor.transpose(
                            pxT[:, s * C + 128:(s + 1) * C], bsub[:, 128:256], ident)
                    xT = xtsb.tile([128, gc], BF16, tag="xT")
                    nc.scalar.copy(out=xT, in_=pxT)
                    # pd = bias + x @ E; bias via two N=512 const outer products
                    pd = psd.tile([128, gc], F32, tag="pd")
                    nc.tensor.matmul(pd[:, 0:512], ones_sc,
                                     bias_sb[:, 0:512],
                                     start=True, stop=False)
                    nc.tensor.matmul(pd[:, 512:1024], ones_sc,
                                     bias_sb[:, 512:1024],
                                     start=True, stop=False)
                    for s in range(GROUP):
                        nc.tensor.matmul(
                            pd[:, s * C:(s + 1) * C], xT[:, s * C:s * C + 128],
                            e_sb[:, 0:C], start=False, stop=False,
                        )
                        nc.tensor.matmul(
                            pd[:, s * C:(s + 1) * C], xT[:, s * C + 128:(s + 1) * C],
                            e_sb[:, C:2 * C], start=False, stop=True,
                        )
                    nc.vector.tensor_add(
                        out=yt[:, g0 * C:g0 * C + gc],
                        in0=xt[:, g0 * C:g0 * C + gc], in1=pd)
                # stores on the idle SWDGE ring; loads keep the sync HWDGE ring
                nc.gpsimd.dma_start(out=_chunk_ap(y, row0, nsub), in_=yt[:, : nsub * C])
                row0 += 128 * nsub
    nc.finalize()
    return nc


_PROGRAMS = {}


def _get_programs():
    if "p1" not in _PROGRAMS:
        _PROGRAMS["p1"] = build_pass1()
        _PROGRAMS["p2"] = build_pass2()
    return _PROGRAMS["p1"], _PROGRAMS["p2"]


def _tri_inv_lower(L):
    try:
        from scipy.linalg import solve_triangular
        return solve_triangular(L, np.eye(C, dtype=L.dtype), lower=True)
    except ImportError:
        return np.linalg.solve(L, np.eye(C, dtype=L.dtype))


def kernel(x, gamma, beta):
    LAST_RESULTS.clear()
    x = np.ascontiguousarray(x, dtype=np.float32)
    gamma = np.asarray(gamma, dtype=np.float64).reshape(C)
    beta = np.asarray(beta, dtype=np.float64).reshape(C)
    xf = x.reshape(N, C)
    nc1, nc2 = _get_programs()
    core_ids = list(range(NCORES))

    in_maps1 = [{"x": xf[i * NL:(i + 1) * NL]} for i in range(NCORES)]
    r1 = run_bass_kernel_spmd(nc1, in_maps1, core_ids=core_ids)
    LAST_RESULTS.append(("gram", r1))

    G = np.zeros((C, C), np.float64)
    for r in r1.results:
        G += r["g"].astype(np.float64)
    m = xf.sum(axis=0, dtype=np.float64) / N
    cov = (G - N * np.outer(m, m)) / (N - 1.0)
    ff = (1.0 - EPS) * cov + EPS * np.eye(C)
    L = np.linalg.cholesky(ff)
    Winv = _tri_inv_lower(L)                     # W = L^-1 (lower)
    A = Winv.T * gamma[None, :]                  # A[i,j] = gamma_j * W[j,i]
    E = np.ascontiguousarray((A - np.eye(C)).astype(ml_dtypes.bfloat16))
    bvec = np.ascontiguousarray(
        (beta - gamma * (Winv @ m)).astype(ml_dtypes.bfloat16).reshape(1, C)
    )

    in_maps2 = [
        {"x": xf[i * NL:(i + 1) * NL], "e": E, "bvec": bvec}
        for i in range(NCORES)
    ]
    r2 = run_bass_kernel_spmd(nc2, in_maps2, core_ids=core_ids)
    LAST_RESULTS.append(("whiten", r2))

    out = np.empty((N, C), np.float32)
    for i, r in enumerate(r2.results):
        out[i * NL:(i + 1) * NL] = r["y"]
    return out.reshape(B, W, H, C)
